# revision 3
# baseline (speedup 1.0000x reference)
"""Trainium2 Bass kernel for nn_Attention (B=2, S=2048, D=1024, H=16, causal).

Sharding: head-parallel across 8 NeuronCores — 2 heads per core. Each core:
  1. computes qT/kT/vT for its 2 heads from the full xT (QKV projection,
     transposed layout [128 = 2*hd, S]),
  2. runs causal attention per head with scores in transposed orientation
     (sT[sj, si]) so the PV matmul needs no P transpose; the softmax
     denominator comes free as an extra ones-column in the V operand,
  3. multiplies by its 128-row slice of W_proj producing a partial output
     yT_c [B, D, S].
Host sums the 8 partials, adds b_proj, and transposes back to [B, S, D].

All matmuls run in float32r (full-rate fp32 on the PE; ~1e-4 rounding).
"""
import sys

sys.path.insert(0, "/opt/trn_rl_repo")

import numpy as np
import concourse.bacc as bacc
import concourse.mybir as mybir
import concourse.tile as tile
from concourse.bass_utils import run_bass_kernel_spmd

dt = mybir.dt
F32R = dt.float32r
AF = mybir.ActivationFunctionType

B, S, D, H = 2, 2048, 1024, 16
HD = D // H            # 64
NCORE = 8
HPC = H // NCORE       # 2 heads per core
NEG = -30000.0         # exp((s + NEG) * 0.125) == 0 in fp32

_CACHE = {}


def build_nc():
    nc = bacc.Bacc("TRN2", target_bir_lowering=False, debug=False)

    xT_d = nc.dram_tensor("xT", [B, D, S], dt.float32, kind="ExternalInput")
    wq_d = nc.dram_tensor("wq", [D, 128], dt.float32, kind="ExternalInput")
    wk_d = nc.dram_tensor("wk", [D, 128], dt.float32, kind="ExternalInput")
    wv_d = nc.dram_tensor("wv", [D, 128], dt.float32, kind="ExternalInput")
    bq_d = nc.dram_tensor("bq", [128, 1], dt.float32, kind="ExternalInput")
    bk_d = nc.dram_tensor("bk", [128, 1], dt.float32, kind="ExternalInput")
    bv_d = nc.dram_tensor("bv", [128, 1], dt.float32, kind="ExternalInput")
    wp_d = nc.dram_tensor("wp", [128, D], dt.float32, kind="ExternalInput")
    negm_d = nc.dram_tensor("negm", [128, 128], dt.float32, kind="ExternalInput")
    id_d = nc.dram_tensor("ident", [128, 128], dt.float32, kind="ExternalInput")
    ones_d = nc.dram_tensor("ones", [128, 64], dt.float32, kind="ExternalInput")
    yT_d = nc.dram_tensor("yT", [B, D, S], dt.float32, kind="ExternalOutput")

    with tile.TileContext(nc) as tc:
        with (
            tc.tile_pool(name="consts", bufs=1) as consts,
            tc.tile_pool(name="xpool", bufs=1) as xpool,
            tc.tile_pool(name="qkv", bufs=1) as qkvp,
            tc.tile_pool(name="epool", bufs=3) as epool,
            tc.tile_pool(name="ypool", bufs=3) as ypool,
            tc.tile_pool(name="rpool", bufs=2) as rpool,
            tc.tile_pool(name="ps_mm", bufs=4, space="PSUM") as ps_mm,
            tc.tile_pool(name="ps_a", bufs=2, space="PSUM") as ps_a_pool,
            tc.tile_pool(name="ps_t", bufs=2, space="PSUM") as ps_t_pool,
        ):
            # ---- constants / weights (once) ----
            wqr = consts.tile([128, 8, 128], F32R, tag="wq")
            wkr = consts.tile([128, 8, 128], F32R, tag="wk")
            wvr = consts.tile([128, 8, 128], F32R, tag="wv")
            for (w_r, w_d) in ((wqr, wq_d), (wkr, wk_d), (wvr, wv_d)):
                for d in range(8):
                    nc.gpsimd.dma_start(
                        w_r[:, d, :], w_d.ap()[128 * d:128 * (d + 1), :]
                    )
            wpr = consts.tile([128, D], F32R, tag="wp")
            nc.gpsimd.dma_start(wpr[:], wp_d.ap()[:])
            bq_sb = consts.tile([128, 1], dt.float32, tag="bq")
            bk_sb = consts.tile([128, 1], dt.float32, tag="bk")
            bv_sb = consts.tile([128, 1], dt.float32, tag="bv")
            nc.sync.dma_start(bq_sb[:], bq_d.ap()[:])
            nc.sync.dma_start(bk_sb[:], bk_d.ap()[:])
            nc.sync.dma_start(bv_sb[:], bv_d.ap()[:])
            negm = consts.tile([128, 128], dt.float32, tag="negm")
            nc.sync.dma_start(negm[:], negm_d.ap()[:])
            ident = consts.tile([128, 128], dt.float32, tag="ident")
            nc.sync.dma_start(ident[:], id_d.ap()[:])
            ones_r = consts.tile([1, 64], F32R, tag="ones")
            nc.gpsimd.dma_start(ones_r[:], ones_d.ap()[0:1, :])

            for b in range(B):
                # ---- load xT[b] as f32r, per d-tile ----
                xr = xpool.tile([128, 8, S], F32R, tag="x")
                for d in range(8):
                    nc.gpsimd.dma_start(
                        xr[:, d, :], xT_d.ap()[b, 128 * d:128 * (d + 1), :]
                    )

                # ---- QKV projection (transposed outputs) ----
                qTr = qkvp.tile([128, S], F32R, tag="qT")
                kTr = qkvp.tile([128, S], F32R, tag="kT")
                vT = qkvp.tile([128, S], dt.float32, tag="vT")
                for (w_r, bias, out_sb) in (
                    (wqr, bq_sb, qTr),
                    (wkr, bk_sb, kTr),
                    (wvr, bv_sb, vT),
                ):
                    for blk in range(4):
                        ps = ps_mm.tile([128, 512], dt.float32, tag="mm")
                        for d in range(8):
                            nc.tensor.matmul(
                                ps[:],
                                w_r[:, d, :],
                                xr[:, d, 512 * blk:512 * (blk + 1)],
                                start=(d == 0),
                                stop=(d == 7),
                            )
                        nc.scalar.activation(
                            out_sb[:, 512 * blk:512 * (blk + 1)], ps[:],
                            AF.Identity, bias=bias[:, 0:1],
                        )

                # ---- vhat: v natural per sj tile + ones column, f32r ----
                # layout [128(sj), 16(j), 130 = 64 v_h0 | 1 | 64 v_h1 | 1]
                vhat = qkvp.tile([128, 16, 130], F32R, tag="vhat")
                nc.gpsimd.dma_start(vhat[:, :, 64], ones_d.ap()[:, 0:16])
                nc.gpsimd.dma_start(vhat[:, :, 129], ones_d.ap()[:, 16:32])
                for j in range(16):
                    pst = ps_t_pool.tile([128, 128], dt.float32, tag="small")
                    nc.tensor.transpose(
                        pst[:], vT[:, 128 * j:128 * (j + 1)], ident[:]
                    )
                    nc.vector.tensor_copy(vhat[:, j, 0:64], pst[:, 0:64])
                    nc.vector.tensor_copy(vhat[:, j, 65:129], pst[:, 64:128])

                # ---- causal attention per head ----
                aT = qkvp.tile([128, S], F32R, tag="aT")
                for hl in range(HPC):
                    p0 = 64 * hl
                    for blk in range(4):
                        si0 = 512 * blk
                        jlast = 4 * blk + 3
                        psa = ps_a_pool.tile([65, 512], dt.float32, tag="acc")
                        for j in range(jlast + 1):
                            off = max(0, 128 * (j - 4 * blk))
                            w = 512 - off
                            pss = ps_mm.tile([128, 512], dt.float32, tag="mm")
                            nc.tensor.matmul(
                                pss[:, 0:w],
                                kTr[p0:p0 + 64, 128 * j:128 * (j + 1)],
                                qTr[p0:p0 + 64, si0 + off:si0 + 512],
                                start=True,
                                stop=True,
                            )
                            if j >= 4 * blk:
                                nc.vector.tensor_add(
                                    pss[:, 0:128], pss[:, 0:128], negm[:]
                                )
                            eT = epool.tile([128, 512], F32R, tag="eT")
                            nc.scalar.activation(
                                eT[:, 0:w], pss[:, 0:w], AF.Exp, scale=0.125
                            )
                            nc.tensor.matmul(
                                psa[:, off:512],
                                vhat[:, j, 65 * hl:65 * hl + 65],
                                eT[:, 0:w],
                                start=(j == 0),
                                stop=(j == jlast),
                            )
                        # normalize: recip of l row, broadcast via K=1 matmul
                        recip = rpool.tile([1, 512], F32R, tag="recip")
                        with nc.allow_low_precision(reason="f32r softmax recip"):
                            nc.vector.reciprocal(recip[:], psa[64:65, :])
                        psb = ps_t_pool.tile([64, 512], dt.float32, tag="small")
                        nc.tensor.matmul(
                            psb[:], ones_r[:], recip[:], start=True, stop=True
                        )
                        bc_sb = rpool.tile([64, 512], dt.float32, tag="bc_sb")
                        nc.vector.tensor_copy(bc_sb[:], psb[:])
                        with nc.allow_low_precision(reason="f32r attn normalize"):
                            nc.vector.tensor_mul(
                                aT[p0:p0 + 64, si0:si0 + 512],
                                psa[0:64, :],
                                bc_sb[:],
                            )

                # ---- output projection (partial over this core's 128 dins) ----
                for dtile in range(8):
                    for blk in range(4):
                        ps = ps_mm.tile([128, 512], dt.float32, tag="mm")
                        nc.tensor.matmul(
                            ps[:],
                            wpr[:, 128 * dtile:128 * (dtile + 1)],
                            aT[:, 512 * blk:512 * (blk + 1)],
                            start=True,
                            stop=True,
                        )
                        y_sb = ypool.tile([128, 512], dt.float32, tag="y")
                        nc.vector.tensor_copy(y_sb[:], ps[:])
                        nc.sync.dma_start(
                            yT_d.ap()[
                                b, 128 * dtile:128 * (dtile + 1),
                                512 * blk:512 * (blk + 1),
                            ],
                            y_sb[:],
                        )
    nc.compile()
    return nc


def _get_nc():
    if "nc" not in _CACHE:
        _CACHE["nc"] = build_nc()
    return _CACHE["nc"]


def make_in_maps(x, W_attn, b_attn, W_proj):
    x = np.ascontiguousarray(x, dtype=np.float32)
    xT = np.ascontiguousarray(x.transpose(0, 2, 1))

    p = np.arange(128)
    negm = np.where(p[:, None] <= p[None, :], 0.0, NEG).astype(np.float32)
    ident = np.eye(128, dtype=np.float32)
    ones = np.ones((128, 64), np.float32)

    in_maps = []
    for c in range(NCORE):
        col0 = HD * HPC * c
        in_maps.append({
            "xT": xT,
            "wq": np.ascontiguousarray(W_attn[:, col0:col0 + 128]),
            "wk": np.ascontiguousarray(W_attn[:, D + col0:D + col0 + 128]),
            "wv": np.ascontiguousarray(W_attn[:, 2 * D + col0:2 * D + col0 + 128]),
            "bq": np.ascontiguousarray(b_attn[col0:col0 + 128].reshape(128, 1)),
            "bk": np.ascontiguousarray(b_attn[D + col0:D + col0 + 128].reshape(128, 1)),
            "bv": np.ascontiguousarray(b_attn[2 * D + col0:2 * D + col0 + 128].reshape(128, 1)),
            "wp": np.ascontiguousarray(W_proj[128 * c:128 * (c + 1), :]),
            "negm": negm,
            "ident": ident,
            "ones": ones,
        })
    return in_maps


def gather(results, b_proj):
    acc = np.zeros((B, D, S), np.float64)
    for r in results:
        acc += r["yT"]
    out = acc.transpose(0, 2, 1) + np.asarray(b_proj, np.float64)[None, None, :]
    return np.ascontiguousarray(out.astype(np.float32))


def kernel(x, W_attn, b_attn, W_proj, b_proj, _trace=False, _trace_kwargs=None):
    nc = _get_nc()
    in_maps = make_in_maps(np.asarray(x), np.asarray(W_attn),
                           np.asarray(b_attn), np.asarray(W_proj))
    res = run_bass_kernel_spmd(
        nc, in_maps, list(range(NCORE)), trace=_trace, **(_trace_kwargs or {})
    )
    out = gather(res.results, np.asarray(b_proj))
    if _trace:
        kernel.last_result = res
    return out
